# revision 39
# baseline (speedup 1.0000x reference)
"""Causal GQA self-attention (B=2, T=2048, D=2048, 16 q-heads / 4 kv-heads,
head_dim=128, full-dim RoPE) on 8 Trainium2 NeuronCores.

Strategy: tensor-parallel over heads. Core c owns q-heads {2c, 2c+1} and
kv-head c//2. Wq/Wkv output dims and Wproj input dims are sharded 8-ways on
the host; each core computes a full-width partial of the output projection
(in bf16) and the host sums the 8 partials.

On-chip layout: x is staged transposed (channel-major [C, B*T]) so the
QKV projections need no on-chip transpose; attention is computed "k-major"
(scores transposed, [k_pos, q_pos]) so the P@V contraction needs no
transpose either. Softmax runs without max-subtraction (scores are ~N(0,1);
exp never overflows). The denominator is accumulated on the DVE (bf16 adds
of the exp tiles) with a single ones-stationary matmul per (chunk, head) to
reduce across partitions — keeping the per-k-tile PE cost at 2 matmuls
(scores + P@V) instead of 3. yT is normalized straight out of PSUM with a
fast approximate reciprocal. Out-projection PSUM banks are drained by the
(otherwise idle) GpSimd engine into bf16 staging buffers.
"""

import math
import os
import sys

for _p in ("/opt/trn_rl_repo", "/root/.axon_site/_ro/trn_rl_repo"):
    if os.path.isdir(_p) and _p not in sys.path:
        sys.path.insert(0, _p)

import ml_dtypes
import numpy as np

BF16 = ml_dtypes.bfloat16

B = 2
T = 2048
C = 2048
D = 128          # head dim
NQH = 2          # q heads per core
TOK = B * T      # 4096
KT = C // 128    # 16 contraction tiles
NCH = 512        # matmul moving-dim chunk
QCH = T // NCH   # 4 q chunks per batch
KB = T // 128    # 16 k tiles per batch
N_CORES = 8
SCALE = 1.0 / math.sqrt(D)

_COMPILED = {}


def _rope_tables():
    dim = np.arange(D // 2, dtype=np.float64)
    freq = 10000.0 ** (dim / (D / 2))
    freq = np.concatenate([freq, freq])              # [128]
    pos = np.arange(T, dtype=np.float64)
    ang = pos[None, :] / freq[:, None]               # [128, T] channel-major
    return np.cos(ang), np.sin(ang)


def _build_nc(debug=False):
    import concourse.bass as bass  # noqa: F401
    import concourse.mybir as mybir
    import concourse.tile as tile
    from concourse import bacc
    from concourse.bass import ts

    f32 = mybir.dt.float32
    bf16 = mybir.dt.bfloat16
    AF = mybir.ActivationFunctionType
    OP = mybir.AluOpType

    nc = bacc.Bacc("TRN2", target_bir_lowering=False, debug=False,
                   num_devices=N_CORES)

    # weights are staged partition-major on the host ([128, KT, n] flattened)
    # so every weight DMA reads fully contiguous rows (~2x DMA throughput vs
    # the 512B-strided (ko p) n -> p ko n gather).
    xt_e = nc.dram_tensor("xt", [C, TOK], bf16, kind="ExternalInput")
    wq_e = nc.dram_tensor("wq", [128, KT * NQH * D], bf16, kind="ExternalInput")
    wk_e = nc.dram_tensor("wk", [128, KT * D], bf16, kind="ExternalInput")
    wv_e = nc.dram_tensor("wv", [128, KT * D], bf16, kind="ExternalInput")
    wp_e = nc.dram_tensor("wp", [128, NQH * C], bf16, kind="ExternalInput")
    cos_e = nc.dram_tensor("cos", [D, T], bf16, kind="ExternalInput")
    sin_e = nc.dram_tensor("sin", [D, T], bf16, kind="ExternalInput")
    tri_e = nc.dram_tensor("tri", [D, D], bf16, kind="ExternalInput")
    out_e = nc.dram_tensor("out", [TOK, C], bf16, kind="ExternalOutput")

    from contextlib import ExitStack

    with tile.TileContext(nc) as tc, ExitStack() as ctx:
        const = ctx.enter_context(tc.tile_pool(name="const", bufs=1))
        qkvp = ctx.enter_context(tc.tile_pool(name="qkv", bufs=1))
        psum = ctx.enter_context(tc.tile_pool(name="ps", bufs=3, space="PSUM"))
        xtp = ctx.enter_context(tc.tile_pool(name="xt", bufs=1))
        w1p = ctx.enter_context(tc.tile_pool(name="w1", bufs=1))
        rtp = ctx.enter_context(tc.tile_pool(name="rt", bufs=3))
        exp_p = ctx.enter_context(tc.tile_pool(name="exp", bufs=6))
        denp = ctx.enter_context(tc.tile_pool(name="den", bufs=2))
        recp = ctx.enter_context(tc.tile_pool(name="rec", bufs=2))
        outp = ctx.enter_context(tc.tile_pool(name="outs", bufs=3))

        # DMA emission order matches phase-1 consumption: wq, then the first
        # 512-token chunk of xt (in 4 half-MB pieces so the first Q chain can
        # start early), then wk/wv/cos/sin. Few large DMAs — the Sync engine
        # serializes DMA issue at ~0.7us each, so per-kt splits starve the PE.
        wq_sb = w1p.tile([128, KT, NQH * D], bf16, tag="wq")
        wq_r = wq_e.ap().rearrange("p (ko n) -> p ko n", ko=KT)
        xt0_sb = xtp.tile([128, KT, T], bf16, tag="xt")
        xt_r = xt_e.ap().rearrange("(ko p) t -> p ko t", p=128)
        # first Q chain starts after wq half 0 + xt quarter 0 (~1MB)
        nc.sync.dma_start(wq_sb[:, 0:8, :], wq_r[:, 0:8, :])
        nc.sync.dma_start(xt0_sb[:, 0:4, 0:NCH], xt_r[:, 0:4, 0:NCH])
        nc.sync.dma_start(wq_sb[:, 8:16, :], wq_r[:, 8:16, :])
        for g in range(1, 4):
            nc.sync.dma_start(xt0_sb[:, 4 * g:4 * g + 4, 0:NCH],
                              xt_r[:, 4 * g:4 * g + 4, 0:NCH])
        wk_sb = w1p.tile([128, KT, D], bf16, tag="wk")
        nc.sync.dma_start(wk_sb[:], wk_e.ap().rearrange("p (ko n) -> p ko n", ko=KT))
        wv_sb = w1p.tile([128, KT, D], bf16, tag="wv")
        nc.sync.dma_start(wv_sb[:], wv_e.ap().rearrange("p (ko n) -> p ko n", ko=KT))
        cos_sb = const.tile([D, T], bf16, tag="cos")
        nc.sync.dma_start(cos_sb[:], cos_e.ap())
        sin_sb = const.tile([D, T], bf16, tag="sin")
        nc.sync.dma_start(sin_sb[:], sin_e.ap())
        tri_sb = const.tile([D, D], bf16, tag="tri")
        nc.sync.dma_start(tri_sb[:], tri_e.ap())
        ones_sb = const.tile([128, 128], bf16, tag="ones")
        nc.vector.memset(ones_sb[:], 1.0)
        wp_sb = const.tile([128, NQH, C], bf16, tag="wp")
        nc.sync.dma_start(wp_sb[:], wp_e.ap().rearrange("p (ko n) -> p ko n", ko=NQH))

        # persistent per-batch-pair tensors
        qT = qkvp.tile([D, NQH, TOK], bf16, tag="qT")    # rope'd, pre-scaled
        kT = qkvp.tile([D, TOK], bf16, tag="kT")         # rope'd
        vv = qkvp.tile([128, B * KB, D], bf16, tag="vv")  # token-major
        yT = qkvp.tile([D, NQH, TOK], bf16, tag="yT")    # attn out, normalized

        def rope_out(dst, src_ps, cos_ap, sin_ap):
            """dst(bf16 sbuf) = src * cos + rotate_half(src) * sin.

            The PSUM result is first cast to bf16 on the ACT engine so every
            DVE op here runs on pure 16-bit operands (2x DVE rate)."""
            sb = rtp.tile([128, NCH], bf16, tag="sb")
            nc.scalar.copy(sb[:], src_ps)
            rt = rtp.tile([128, NCH], bf16, tag="rt")
            nc.vector.tensor_scalar(out=rt[0:64, :], in0=sb[64:128, :],
                                    scalar1=-1.0, scalar2=None, op0=OP.mult)
            nc.vector.tensor_copy(out=rt[64:128, :], in_=sb[0:64, :])
            m1 = rtp.tile([128, NCH], bf16, tag="m1")
            nc.vector.tensor_tensor(out=m1[:], in0=sb[:], in1=cos_ap, op=OP.mult)
            m2 = rtp.tile([128, NCH], bf16, tag="m2")
            nc.vector.tensor_tensor(out=m2[:], in0=rt[:], in1=sin_ap, op=OP.mult)
            nc.vector.tensor_tensor(out=dst, in0=m1[:], in1=m2[:], op=OP.add)

        def emit_proj(b, qc, act_frac=2, final=False):
            # act_frac: how many of each qt-tile's 4 PSUM drains run on the
            # ACT engine (rest on DVE). Tuned per emission window so neither
            # engine outruns its exp/denominator budget. final: the very last
            # proj — both vector engines idle; drain eagerly and overlap the
            # output DMAs (issued from two queues) with remaining matmuls.
            tok0 = b * T
            for qt in range(4 * qc, 4 * qc + 4):
                osb = outp.tile([128, C], bf16, tag="osb")
                for fc in range(C // NCH):
                    ops = psum.tile([128, NCH], f32, tag="mm")
                    for kd in range(NQH):
                        nc.tensor.matmul(
                            ops[:],
                            yT[:, kd, tok0 + qt * 128: tok0 + (qt + 1) * 128],
                            wp_sb[:, kd, ts(fc, NCH)],
                            start=(kd == 0), stop=(kd == NQH - 1))
                    act_set = {0: (), 1: (0,), 2: (0, 2), 3: (0, 1, 2),
                               4: (0, 1, 2, 3)}[act_frac]
                    if fc in act_set:
                        nc.scalar.copy(osb[:, ts(fc, NCH)], ops[:])
                    else:
                        nc.vector.tensor_copy(osb[:, ts(fc, NCH)], ops[:])
                    if final:
                        eng = nc.sync if fc % 2 == 0 else nc.gpsimd
                        eng.dma_start(
                            out_e.ap()[tok0 + qt * 128: tok0 + (qt + 1) * 128,
                                       ts(fc, NCH)],
                            osb[:, ts(fc, NCH)])
                if not final:
                    nc.sync.dma_start(
                        out_e.ap()[tok0 + qt * 128: tok0 + (qt + 1) * 128, :],
                        osb[:])

        pending = None
        for b in range(B):
            tok0 = b * T
            # ---- phase 1: QKV projection + RoPE for batch b ----
            # chunk-major loads so the first projections start after ~2MB
            if b == 0:
                xt_sb = xt0_sb  # chunk 0 DMAs already emitted up top
                first_tc = 1
            else:
                xt_sb = xtp.tile([128, KT, T], bf16, tag="xt")
                first_tc = 0
            for tc_ in range(first_tc, QCH):
                for g in range(4):
                    nc.sync.dma_start(
                        xt_sb[:, 4 * g:4 * g + 4, ts(tc_, NCH)],
                        xt_r[:, 4 * g:4 * g + 4,
                             tok0 + tc_ * NCH: tok0 + (tc_ + 1) * NCH])
            for tc_ in range(QCH):
                for h in range(NQH):
                    ps = psum.tile([128, NCH], f32, tag="mm")
                    for kt in range(KT):
                        nc.tensor.matmul(ps[:],
                                         wq_sb[:, kt, h * D:(h + 1) * D],
                                         xt_sb[:, kt, ts(tc_, NCH)],
                                         start=(kt == 0), stop=(kt == KT - 1))
                    rope_out(qT[:, h, tok0 + tc_ * NCH: tok0 + (tc_ + 1) * NCH],
                             ps, cos_sb[:, ts(tc_, NCH)], sin_sb[:, ts(tc_, NCH)])
                ps = psum.tile([128, NCH], f32, tag="mm")
                for kt in range(KT):
                    nc.tensor.matmul(ps[:], wk_sb[:, kt, :],
                                     xt_sb[:, kt, ts(tc_, NCH)],
                                     start=(kt == 0), stop=(kt == KT - 1))
                rope_out(kT[:, tok0 + tc_ * NCH: tok0 + (tc_ + 1) * NCH],
                         ps, cos_sb[:, ts(tc_, NCH)], sin_sb[:, ts(tc_, NCH)])
                for ti in range(4 * tc_, 4 * tc_ + 4):
                    ps = psum.tile([128, D], f32, tag="mm")
                    for kt in range(KT):
                        nc.tensor.matmul(ps[:],
                                         xt_sb[:, kt, ti * 128:(ti + 1) * 128],
                                         wv_sb[:, kt, :],
                                         start=(kt == 0), stop=(kt == KT - 1))
                    nc.scalar.copy(vv[:, b * KB + ti, :], ps[:])

            # ---- phase 2+3: attention + out-projection for batch b ----
            # proj emission is delayed one chunk so the PE stream always has
            # the next attention chunk ahead of each proj (hides the
            # reciprocal->normalize chain on DVE).
            for qc in range(QCH):
                for h in range(NQH):
                    n_kt = 4 * qc + 4
                    yps = psum.tile([128, NCH], f32, tag="y", bufs=2)
                    den = denp.tile([128, NCH], bf16, tag="den")
                    for kti in range(n_kt):
                        dq = kti - 4 * qc
                        c0 = dq * 128 if dq > 0 else 0  # masked cols skipped
                        q_sl = qT[:, h, tok0 + qc * NCH + c0:
                                  tok0 + (qc + 1) * NCH]
                        sc = psum.tile([128, NCH], f32, tag="sc")
                        nc.tensor.matmul(sc[:, c0:],
                                         kT[:, tok0 + kti * 128: tok0 + (kti + 1) * 128],
                                         q_sl, start=True, stop=True)
                        ex = exp_p.tile([128, NCH], bf16, tag="ex")
                        nc.scalar.activation(ex[:, c0:], sc[:, c0:], AF.Exp)
                        if dq >= 0:
                            # causal mask on the (otherwise idle) Pool engine;
                            # the last two diagonals run on DVE so the
                            # denominator reduce (whose DVE adds are serial)
                            # isn't gated on a Pool round-trip
                            eng_tri = nc.vector if dq >= 2 else nc.gpsimd
                            eng_tri.tensor_mul(ex[:, ts(dq, 128)],
                                               ex[:, ts(dq, 128)], tri_sb[:])
                        # denominator partial-sum on DVE (bf16, 2x rate)
                        if kti == 0:
                            nc.vector.tensor_copy(den[:], ex[:])
                        else:
                            nc.vector.tensor_tensor(out=den[:, c0:],
                                                    in0=den[:, c0:],
                                                    in1=ex[:, c0:], op=OP.add)
                        nc.tensor.matmul(yps[:, c0:], vv[:, b * KB + kti, :],
                                         ex[:, c0:],
                                         start=(kti == 0), stop=(kti == n_kt - 1))
                    # reduce denominator across partitions (one matmul), then
                    # normalize yT straight out of PSUM on the DVE.
                    dps = psum.tile([128, NCH], f32, tag="sc")
                    nc.tensor.matmul(dps[:], ones_sb[:], den[:],
                                     start=True, stop=True)
                    rec = recp.tile([128, NCH], f32, tag="rec")
                    nc.vector.reciprocal_approx_fast(rec[:], dps[:])
                    nc.vector.tensor_mul(
                        yT[:, h, tok0 + qc * NCH: tok0 + (qc + 1) * NCH],
                        yps[:], rec[:])
                if pending is not None:
                    # ACT share of PSUM drains per window: exp load grows with
                    # qc, so shift drains toward DVE in later windows.
                    emit_proj(*pending, act_frac=(2, 2, 1, 0)[qc])
                pending = (b, qc)
        emit_proj(*pending, act_frac=2, final=True)

    nc.compile()
    return nc


def _get_nc():
    if "nc" not in _COMPILED:
        _COMPILED["nc"] = _build_nc()
    return _COMPILED["nc"]


def _stage_inputs(x, Wq, Wkv, Wproj):
    xt = np.ascontiguousarray(
        x.reshape(TOK, C).T).astype(BF16)                       # [C, TOK]
    cos, sin = _rope_tables()
    cos = cos.astype(BF16)
    sin = sin.astype(BF16)
    kk, qq = np.meshgrid(np.arange(D), np.arange(D), indexing="ij")
    tri = (kk <= qq).astype(BF16)                               # [k, q]

    def pmaj(w):
        # [k, n] -> partition-major [128, (k//128)*n] so device DMAs read
        # fully contiguous rows
        k, n = w.shape
        return np.ascontiguousarray(
            w.reshape(k // 128, 128, n).transpose(1, 0, 2)
            .reshape(128, (k // 128) * n))

    in_maps = []
    for c in range(N_CORES):
        g = c // 2
        wq = pmaj((Wq[2 * c * D:(2 * c + 2) * D, :] * SCALE).T.astype(BF16))
        wk = pmaj(Wkv[g * D:(g + 1) * D, :].T.astype(BF16))
        wv = pmaj(Wkv[4 * D + g * D: 4 * D + (g + 1) * D, :].T.astype(BF16))
        wp = pmaj(Wproj[:, 2 * c * D:(2 * c + 2) * D].T.astype(BF16))
        in_maps.append({
            "xt": xt, "wq": wq, "wk": wk, "wv": wv, "wp": wp,
            "cos": cos, "sin": sin, "tri": tri,
        })
    return in_maps


def run(x, Wq, Wkv, Wproj, trace=False):
    from concourse.bass_utils import run_bass_kernel_spmd

    nc = _get_nc()
    in_maps = _stage_inputs(x, Wq, Wkv, Wproj)
    res = run_bass_kernel_spmd(nc, in_maps, core_ids=list(range(N_CORES)),
                               trace=trace)
    acc = np.zeros((TOK, C), np.float32)
    for c in range(N_CORES):
        acc += res.results[c]["out"].astype(np.float32)
    out = acc.reshape(B, T, C)
    return (out, res) if trace else (out, None)


def kernel(x, Wq, Wkv, Wproj):
    out, _ = run(np.asarray(x, np.float32), np.asarray(Wq, np.float32),
                 np.asarray(Wkv, np.float32), np.asarray(Wproj, np.float32))
    return out


# revision 41
# speedup vs baseline: 1.1859x; 1.1859x over previous
"""Causal GQA self-attention (B=2, T=2048, D=2048, 16 q-heads / 4 kv-heads,
head_dim=128, full-dim RoPE) on 8 Trainium2 NeuronCores.

Strategy: tensor-parallel over heads. Core c owns q-heads {2c, 2c+1} and
kv-head c//2. Wq/Wkv output dims and Wproj input dims are sharded 8-ways on
the host; each core computes a full-width partial of the output projection
(in bf16) and the host sums the 8 partials.

On-chip layout: x is staged transposed (channel-major [C, B*T]) so the
QKV projections need no on-chip transpose; attention is computed "k-major"
(scores transposed, [k_pos, q_pos]) so the P@V contraction needs no
transpose either. Softmax runs without max-subtraction (scores are ~N(0,1);
exp never overflows). The denominator is accumulated on the DVE (bf16 adds
of the exp tiles) with a single ones-stationary matmul per (chunk, head) to
reduce across partitions — keeping the per-k-tile PE cost at 2 matmuls
(scores + P@V) instead of 3. yT is normalized straight out of PSUM with a
fast approximate reciprocal. Out-projection PSUM banks are drained by the
(otherwise idle) GpSimd engine into bf16 staging buffers.
"""

import math
import os
import sys

for _p in ("/opt/trn_rl_repo", "/root/.axon_site/_ro/trn_rl_repo"):
    if os.path.isdir(_p) and _p not in sys.path:
        sys.path.insert(0, _p)

import ml_dtypes
import numpy as np

BF16 = ml_dtypes.bfloat16

B = 2
T = 2048
C = 2048
D = 128          # head dim
NQH = 2          # q heads per core
TOK = B * T      # 4096
KT = C // 128    # 16 contraction tiles
NCH = 512        # matmul moving-dim chunk
QCH = T // NCH   # 4 q chunks per batch
KB = T // 128    # 16 k tiles per batch
N_CORES = 8
SCALE = 1.0 / math.sqrt(D)

_COMPILED = {}


def _rope_tables():
    dim = np.arange(D // 2, dtype=np.float64)
    freq = 10000.0 ** (dim / (D / 2))
    freq = np.concatenate([freq, freq])              # [128]
    pos = np.arange(T, dtype=np.float64)
    ang = pos[None, :] / freq[:, None]               # [128, T] channel-major
    return np.cos(ang), np.sin(ang)


def _build_nc(debug=False):
    import concourse.bass as bass  # noqa: F401
    import concourse.mybir as mybir
    import concourse.tile as tile
    from concourse import bacc
    from concourse.bass import ts

    f32 = mybir.dt.float32
    bf16 = mybir.dt.bfloat16
    AF = mybir.ActivationFunctionType
    OP = mybir.AluOpType

    nc = bacc.Bacc("TRN2", target_bir_lowering=False, debug=False,
                   num_devices=N_CORES)

    # weights are staged partition-major on the host ([128, KT, n] flattened)
    # so every weight DMA reads fully contiguous rows (~2x DMA throughput vs
    # the 512B-strided (ko p) n -> p ko n gather).
    xt_e = nc.dram_tensor("xt", [C, TOK], bf16, kind="ExternalInput")
    wq_e = nc.dram_tensor("wq", [128, KT * NQH * D], bf16, kind="ExternalInput")
    wk_e = nc.dram_tensor("wk", [128, KT * D], bf16, kind="ExternalInput")
    wv_e = nc.dram_tensor("wv", [128, KT * D], bf16, kind="ExternalInput")
    wp_e = nc.dram_tensor("wp", [128, NQH * C], bf16, kind="ExternalInput")
    cos_e = nc.dram_tensor("cos", [D, T], bf16, kind="ExternalInput")
    sin_e = nc.dram_tensor("sin", [D, T], bf16, kind="ExternalInput")
    tri_e = nc.dram_tensor("tri", [D, D], bf16, kind="ExternalInput")
    out_e = nc.dram_tensor("out", [TOK, C], bf16, kind="ExternalOutput")

    from contextlib import ExitStack

    with tile.TileContext(nc) as tc, ExitStack() as ctx:
        const = ctx.enter_context(tc.tile_pool(name="const", bufs=1))
        qkvp = ctx.enter_context(tc.tile_pool(name="qkv", bufs=1))
        psum = ctx.enter_context(tc.tile_pool(name="ps", bufs=3, space="PSUM"))
        xtp = ctx.enter_context(tc.tile_pool(name="xt", bufs=1))
        w1p = ctx.enter_context(tc.tile_pool(name="w1", bufs=1))
        rtp = ctx.enter_context(tc.tile_pool(name="rt", bufs=3))
        exp_p = ctx.enter_context(tc.tile_pool(name="exp", bufs=6))
        denp = ctx.enter_context(tc.tile_pool(name="den", bufs=2))
        recp = ctx.enter_context(tc.tile_pool(name="rec", bufs=2))
        outp = ctx.enter_context(tc.tile_pool(name="outs", bufs=3))

        # DMA emission order matches phase-1 consumption: wq, then the first
        # 512-token chunk of xt (in 4 half-MB pieces so the first Q chain can
        # start early), then wk/wv/cos/sin. Few large DMAs — the Sync engine
        # serializes DMA issue at ~0.7us each, so per-kt splits starve the PE.
        wq_sb = w1p.tile([128, KT, NQH * D], bf16, tag="wq")
        wq_r = wq_e.ap().rearrange("p (ko n) -> p ko n", ko=KT)
        xt0_sb = xtp.tile([128, KT, T], bf16, tag="xt")
        xt_r = xt_e.ap().rearrange("(ko p) t -> p ko t", p=128)
        # first Q chain starts after wq half 0 + xt quarter 0 (~1MB)
        nc.sync.dma_start(wq_sb[:, 0:8, :], wq_r[:, 0:8, :])
        nc.sync.dma_start(xt0_sb[:, 0:4, 0:NCH], xt_r[:, 0:4, 0:NCH])
        nc.sync.dma_start(wq_sb[:, 8:16, :], wq_r[:, 8:16, :])
        for g in range(1, 4):
            nc.sync.dma_start(xt0_sb[:, 4 * g:4 * g + 4, 0:NCH],
                              xt_r[:, 4 * g:4 * g + 4, 0:NCH])
        wk_sb = w1p.tile([128, KT, D], bf16, tag="wk")
        nc.sync.dma_start(wk_sb[:], wk_e.ap().rearrange("p (ko n) -> p ko n", ko=KT))
        wv_sb = w1p.tile([128, KT, D], bf16, tag="wv")
        nc.sync.dma_start(wv_sb[:], wv_e.ap().rearrange("p (ko n) -> p ko n", ko=KT))
        cos_sb = const.tile([D, T], bf16, tag="cos")
        nc.sync.dma_start(cos_sb[:], cos_e.ap())
        sin_sb = const.tile([D, T], bf16, tag="sin")
        nc.sync.dma_start(sin_sb[:], sin_e.ap())
        tri_sb = const.tile([D, D], bf16, tag="tri")
        nc.sync.dma_start(tri_sb[:], tri_e.ap())
        ones_sb = const.tile([128, 128], bf16, tag="ones")
        nc.vector.memset(ones_sb[:], 1.0)
        wp_sb = const.tile([128, NQH, C], bf16, tag="wp")
        nc.sync.dma_start(wp_sb[:], wp_e.ap().rearrange("p (ko n) -> p ko n", ko=NQH))

        # persistent per-batch-pair tensors
        qT = qkvp.tile([D, NQH, TOK], bf16, tag="qT")    # rope'd, pre-scaled
        kT = qkvp.tile([D, TOK], bf16, tag="kT")         # rope'd
        vv = qkvp.tile([128, B * KB, D], bf16, tag="vv")  # token-major
        yT = qkvp.tile([D, NQH, TOK], bf16, tag="yT")    # attn out, normalized

        def rope_out(dst, src_ps, cos_ap, sin_ap):
            """dst(bf16 sbuf) = src * cos + rotate_half(src) * sin.

            The PSUM result is first cast to bf16 on the ACT engine so every
            DVE op here runs on pure 16-bit operands (2x DVE rate)."""
            sb = rtp.tile([128, NCH], bf16, tag="sb")
            nc.scalar.copy(sb[:], src_ps)
            rt = rtp.tile([128, NCH], bf16, tag="rt")
            nc.vector.tensor_scalar(out=rt[0:64, :], in0=sb[64:128, :],
                                    scalar1=-1.0, scalar2=None, op0=OP.mult)
            nc.vector.tensor_copy(out=rt[64:128, :], in_=sb[0:64, :])
            m1 = rtp.tile([128, NCH], bf16, tag="m1")
            nc.vector.tensor_tensor(out=m1[:], in0=sb[:], in1=cos_ap, op=OP.mult)
            m2 = rtp.tile([128, NCH], bf16, tag="m2")
            nc.vector.tensor_tensor(out=m2[:], in0=rt[:], in1=sin_ap, op=OP.mult)
            nc.vector.tensor_tensor(out=dst, in0=m1[:], in1=m2[:], op=OP.add)

        def emit_proj(b, qc, act_frac=2, final=False):
            # act_frac: how many of each qt-tile's 4 PSUM drains run on the
            # ACT engine (rest on DVE). Tuned per emission window so neither
            # engine outruns its exp/denominator budget. final: the very last
            # proj — both vector engines idle; drain eagerly and overlap the
            # output DMAs (issued from two queues) with remaining matmuls.
            tok0 = b * T
            for qt in range(4 * qc, 4 * qc + 4):
                osb = outp.tile([128, C], bf16, tag="osb")
                for fc in range(C // NCH):
                    ops = psum.tile([128, NCH], f32, tag="mm")
                    for kd in range(NQH):
                        nc.tensor.matmul(
                            ops[:],
                            yT[:, kd, tok0 + qt * 128: tok0 + (qt + 1) * 128],
                            wp_sb[:, kd, ts(fc, NCH)],
                            start=(kd == 0), stop=(kd == NQH - 1))
                    act_set = {0: (), 1: (0,), 2: (0, 2), 3: (0, 1, 2),
                               4: (0, 1, 2, 3)}[act_frac]
                    if fc in act_set:
                        nc.scalar.copy(osb[:, ts(fc, NCH)], ops[:])
                    else:
                        nc.vector.tensor_copy(osb[:, ts(fc, NCH)], ops[:])
                    if final:
                        eng = nc.sync if fc % 2 == 0 else nc.gpsimd
                        eng.dma_start(
                            out_e.ap()[tok0 + qt * 128: tok0 + (qt + 1) * 128,
                                       ts(fc, NCH)],
                            osb[:, ts(fc, NCH)])
                if not final:
                    nc.sync.dma_start(
                        out_e.ap()[tok0 + qt * 128: tok0 + (qt + 1) * 128, :],
                        osb[:])

        pending = None
        for b in range(B):
            tok0 = b * T
            # ---- phase 1: QKV projection + RoPE for batch b ----
            # chunk-major loads so the first projections start after ~2MB
            if b == 0:
                xt_sb = xt0_sb  # chunk 0 DMAs already emitted up top
                first_tc = 1
            else:
                xt_sb = xtp.tile([128, KT, T], bf16, tag="xt")
                first_tc = 0
            for tc_ in range(first_tc, QCH):
                for g in range(4):
                    nc.sync.dma_start(
                        xt_sb[:, 4 * g:4 * g + 4, ts(tc_, NCH)],
                        xt_r[:, 4 * g:4 * g + 4,
                             tok0 + tc_ * NCH: tok0 + (tc_ + 1) * NCH])
            for tc_ in range(QCH):
                for h in range(NQH):
                    ps = psum.tile([128, NCH], f32, tag="mm")
                    for kt in range(KT):
                        nc.tensor.matmul(ps[:],
                                         wq_sb[:, kt, h * D:(h + 1) * D],
                                         xt_sb[:, kt, ts(tc_, NCH)],
                                         start=(kt == 0), stop=(kt == KT - 1))
                    rope_out(qT[:, h, tok0 + tc_ * NCH: tok0 + (tc_ + 1) * NCH],
                             ps, cos_sb[:, ts(tc_, NCH)], sin_sb[:, ts(tc_, NCH)])
                ps = psum.tile([128, NCH], f32, tag="mm")
                for kt in range(KT):
                    nc.tensor.matmul(ps[:], wk_sb[:, kt, :],
                                     xt_sb[:, kt, ts(tc_, NCH)],
                                     start=(kt == 0), stop=(kt == KT - 1))
                rope_out(kT[:, tok0 + tc_ * NCH: tok0 + (tc_ + 1) * NCH],
                         ps, cos_sb[:, ts(tc_, NCH)], sin_sb[:, ts(tc_, NCH)])
                for ti in range(4 * tc_, 4 * tc_ + 4):
                    ps = psum.tile([128, D], f32, tag="mm")
                    for kt in range(KT):
                        nc.tensor.matmul(ps[:],
                                         xt_sb[:, kt, ti * 128:(ti + 1) * 128],
                                         wv_sb[:, kt, :],
                                         start=(kt == 0), stop=(kt == KT - 1))
                    nc.scalar.copy(vv[:, b * KB + ti, :], ps[:])

            # ---- phase 2+3: attention + out-projection for batch b ----
            # proj emission is delayed one chunk so the PE stream always has
            # the next attention chunk ahead of each proj (hides the
            # reciprocal->normalize chain on DVE).
            for qc in range(QCH):
                for h in range(NQH):
                    n_kt = 4 * qc + 4
                    yps = psum.tile([128, NCH], f32, tag="y", bufs=2)
                    den = denp.tile([128, NCH], bf16, tag="den")
                    for kti in range(n_kt):
                        dq = kti - 4 * qc
                        c0 = dq * 128 if dq > 0 else 0  # masked cols skipped
                        q_sl = qT[:, h, tok0 + qc * NCH + c0:
                                  tok0 + (qc + 1) * NCH]
                        sc = psum.tile([128, NCH], f32, tag="sc")
                        nc.tensor.matmul(sc[:, c0:],
                                         kT[:, tok0 + kti * 128: tok0 + (kti + 1) * 128],
                                         q_sl, start=True, stop=True)
                        ex = exp_p.tile([128, NCH], bf16, tag="ex")
                        nc.scalar.activation(ex[:, c0:], sc[:, c0:], AF.Exp)
                        if dq >= 0:
                            # causal mask on the (otherwise idle) Pool engine;
                            # the last two diagonals run on DVE so the
                            # denominator reduce (whose DVE adds are serial)
                            # isn't gated on a Pool round-trip
                            eng_tri = nc.vector if dq >= 2 else nc.gpsimd
                            eng_tri.tensor_mul(ex[:, ts(dq, 128)],
                                               ex[:, ts(dq, 128)], tri_sb[:])
                        # denominator partial-sum on DVE (bf16, 2x rate)
                        if kti == 0:
                            nc.vector.tensor_copy(den[:], ex[:])
                        else:
                            nc.vector.tensor_tensor(out=den[:, c0:],
                                                    in0=den[:, c0:],
                                                    in1=ex[:, c0:], op=OP.add)
                        nc.tensor.matmul(yps[:, c0:], vv[:, b * KB + kti, :],
                                         ex[:, c0:],
                                         start=(kti == 0), stop=(kti == n_kt - 1))
                    # reduce denominator across partitions (one matmul), then
                    # normalize yT straight out of PSUM on the DVE.
                    dps = psum.tile([128, NCH], f32, tag="sc")
                    nc.tensor.matmul(dps[:], ones_sb[:], den[:],
                                     start=True, stop=True)
                    rec = recp.tile([128, NCH], f32, tag="rec")
                    nc.vector.reciprocal_approx_fast(rec[:], dps[:])
                    nc.vector.tensor_mul(
                        yT[:, h, tok0 + qc * NCH: tok0 + (qc + 1) * NCH],
                        yps[:], rec[:])
                if pending is not None:
                    # ACT share of PSUM drains per window: exp load grows with
                    # qc, so shift drains toward DVE in later windows.
                    emit_proj(*pending, act_frac=(2, 2, 1, 0)[qc])
                pending = (b, qc)
        emit_proj(*pending, act_frac=2, final=True)

    nc.compile()
    return nc


def _get_nc():
    if "nc" not in _COMPILED:
        _COMPILED["nc"] = _build_nc()
    return _COMPILED["nc"]


def _stage_inputs(x, Wq, Wkv, Wproj):
    xt = np.ascontiguousarray(
        x.reshape(TOK, C).T).astype(BF16)                       # [C, TOK]
    cos, sin = _rope_tables()
    cos = cos.astype(BF16)
    sin = sin.astype(BF16)
    kk, qq = np.meshgrid(np.arange(D), np.arange(D), indexing="ij")
    tri = (kk <= qq).astype(BF16)                               # [k, q]

    def pmaj(w):
        # [k, n] -> partition-major [128, (k//128)*n] so device DMAs read
        # fully contiguous rows
        k, n = w.shape
        return np.ascontiguousarray(
            w.reshape(k // 128, 128, n).transpose(1, 0, 2)
            .reshape(128, (k // 128) * n))

    in_maps = []
    for c in range(N_CORES):
        g = c // 2
        wq = pmaj((Wq[2 * c * D:(2 * c + 2) * D, :] * SCALE).T.astype(BF16))
        wk = pmaj(Wkv[g * D:(g + 1) * D, :].T.astype(BF16))
        wv = pmaj(Wkv[4 * D + g * D: 4 * D + (g + 1) * D, :].T.astype(BF16))
        wp = pmaj(Wproj[:, 2 * c * D:(2 * c + 2) * D].T.astype(BF16))
        in_maps.append({
            "xt": xt, "wq": wq, "wk": wk, "wv": wv, "wp": wp,
            "cos": cos, "sin": sin, "tri": tri,
        })
    return in_maps


def run(x, Wq, Wkv, Wproj, trace=False):
    from concourse.bass_utils import run_bass_kernel_spmd

    nc = _get_nc()
    in_maps = _stage_inputs(x, Wq, Wkv, Wproj)
    res = run_bass_kernel_spmd(nc, in_maps, core_ids=list(range(N_CORES)),
                               trace=trace)
    acc = np.zeros((TOK, C), np.float32)
    for c in range(N_CORES):
        acc += res.results[c]["out"].astype(np.float32)
    out = acc.reshape(B, T, C)
    return (out, res) if trace else (out, None)


def kernel(x, Wq, Wkv, Wproj):
    out, _ = run(np.asarray(x, np.float32), np.asarray(Wq, np.float32),
                 np.asarray(Wkv, np.float32), np.asarray(Wproj, np.float32))
    return out


# revision 42
# speedup vs baseline: 1.1923x; 1.0055x over previous
"""Causal GQA self-attention (B=2, T=2048, D=2048, 16 q-heads / 4 kv-heads,
head_dim=128, full-dim RoPE) on 8 Trainium2 NeuronCores.

Strategy: tensor-parallel over heads. Core c owns q-heads {2c, 2c+1} and
kv-head c//2. Wq/Wkv output dims and Wproj input dims are sharded 8-ways on
the host; each core computes a full-width partial of the output projection
(in bf16) and the host sums the 8 partials.

On-chip layout: x is staged transposed (channel-major [C, B*T]) so the
QKV projections need no on-chip transpose; attention is computed "k-major"
(scores transposed, [k_pos, q_pos]) so the P@V contraction needs no
transpose either. Softmax runs without max-subtraction (scores are ~N(0,1);
exp never overflows). The denominator is accumulated on the DVE (bf16 adds
of the exp tiles) with a single ones-stationary matmul per (chunk, head) to
reduce across partitions — keeping the per-k-tile PE cost at 2 matmuls
(scores + P@V) instead of 3. yT is normalized straight out of PSUM with a
fast approximate reciprocal. Out-projection PSUM banks are drained by the
(otherwise idle) GpSimd engine into bf16 staging buffers.
"""

import math
import os
import sys

for _p in ("/opt/trn_rl_repo", "/root/.axon_site/_ro/trn_rl_repo"):
    if os.path.isdir(_p) and _p not in sys.path:
        sys.path.insert(0, _p)

import ml_dtypes
import numpy as np

BF16 = ml_dtypes.bfloat16

B = 2
T = 2048
C = 2048
D = 128          # head dim
NQH = 2          # q heads per core
TOK = B * T      # 4096
KT = C // 128    # 16 contraction tiles
NCH = 512        # matmul moving-dim chunk
QCH = T // NCH   # 4 q chunks per batch
KB = T // 128    # 16 k tiles per batch
N_CORES = 8
SCALE = 1.0 / math.sqrt(D)

_COMPILED = {}


def _rope_tables():
    dim = np.arange(D // 2, dtype=np.float64)
    freq = 10000.0 ** (dim / (D / 2))
    freq = np.concatenate([freq, freq])              # [128]
    pos = np.arange(T, dtype=np.float64)
    ang = pos[None, :] / freq[:, None]               # [128, T] channel-major
    return np.cos(ang), np.sin(ang)


def _build_nc(debug=False):
    import concourse.bass as bass  # noqa: F401
    import concourse.mybir as mybir
    import concourse.tile as tile
    from concourse import bacc
    from concourse.bass import ts

    f32 = mybir.dt.float32
    bf16 = mybir.dt.bfloat16
    AF = mybir.ActivationFunctionType
    OP = mybir.AluOpType

    nc = bacc.Bacc("TRN2", target_bir_lowering=False, debug=False,
                   num_devices=N_CORES)

    # weights are staged partition-major on the host ([128, KT, n] flattened)
    # so every weight DMA reads fully contiguous rows (~2x DMA throughput vs
    # the 512B-strided (ko p) n -> p ko n gather).
    xt_e = nc.dram_tensor("xt", [C, TOK], bf16, kind="ExternalInput")
    wq_e = nc.dram_tensor("wq", [128, KT * NQH * D], bf16, kind="ExternalInput")
    wk_e = nc.dram_tensor("wk", [128, KT * D], bf16, kind="ExternalInput")
    wv_e = nc.dram_tensor("wv", [128, KT * D], bf16, kind="ExternalInput")
    wp_e = nc.dram_tensor("wp", [128, NQH * C], bf16, kind="ExternalInput")
    cos_e = nc.dram_tensor("cos", [D, T], bf16, kind="ExternalInput")
    sin_e = nc.dram_tensor("sin", [D, T], bf16, kind="ExternalInput")
    tri_e = nc.dram_tensor("tri", [D, D], bf16, kind="ExternalInput")
    out_e = nc.dram_tensor("out", [TOK, C], bf16, kind="ExternalOutput")

    from contextlib import ExitStack

    with tile.TileContext(nc) as tc, ExitStack() as ctx:
        const = ctx.enter_context(tc.tile_pool(name="const", bufs=1))
        qkvp = ctx.enter_context(tc.tile_pool(name="qkv", bufs=1))
        psum = ctx.enter_context(tc.tile_pool(name="ps", bufs=3, space="PSUM"))
        xtp = ctx.enter_context(tc.tile_pool(name="xt", bufs=1))
        w1p = ctx.enter_context(tc.tile_pool(name="w1", bufs=1))
        rtp = ctx.enter_context(tc.tile_pool(name="rt", bufs=3))
        exp_p = ctx.enter_context(tc.tile_pool(name="exp", bufs=6))
        denp = ctx.enter_context(tc.tile_pool(name="den", bufs=2))
        recp = ctx.enter_context(tc.tile_pool(name="rec", bufs=2))
        outp = ctx.enter_context(tc.tile_pool(name="outs", bufs=3))

        # DMA emission order matches phase-1 consumption: wq, then the first
        # 512-token chunk of xt (in 4 half-MB pieces so the first Q chain can
        # start early), then wk/wv/cos/sin. Few large DMAs — the Sync engine
        # serializes DMA issue at ~0.7us each, so per-kt splits starve the PE.
        wq_sb = w1p.tile([128, KT, NQH * D], bf16, tag="wq")
        wq_r = wq_e.ap().rearrange("p (ko n) -> p ko n", ko=KT)
        xt0_sb = xtp.tile([128, KT, T], bf16, tag="xt")
        xt_r = xt_e.ap().rearrange("(ko p) t -> p ko t", p=128)
        # first Q chain starts after wq half 0 + xt quarter 0 (~1MB)
        nc.sync.dma_start(wq_sb[:, 0:8, :], wq_r[:, 0:8, :])
        nc.sync.dma_start(xt0_sb[:, 0:4, 0:NCH], xt_r[:, 0:4, 0:NCH])
        nc.sync.dma_start(wq_sb[:, 8:16, :], wq_r[:, 8:16, :])
        for g in range(1, 4):
            nc.sync.dma_start(xt0_sb[:, 4 * g:4 * g + 4, 0:NCH],
                              xt_r[:, 4 * g:4 * g + 4, 0:NCH])
        wk_sb = w1p.tile([128, KT, D], bf16, tag="wk")
        nc.sync.dma_start(wk_sb[:], wk_e.ap().rearrange("p (ko n) -> p ko n", ko=KT))
        wv_sb = w1p.tile([128, KT, D], bf16, tag="wv")
        nc.sync.dma_start(wv_sb[:], wv_e.ap().rearrange("p (ko n) -> p ko n", ko=KT))
        cos_sb = const.tile([D, T], bf16, tag="cos")
        nc.sync.dma_start(cos_sb[:], cos_e.ap())
        sin_sb = const.tile([D, T], bf16, tag="sin")
        nc.sync.dma_start(sin_sb[:], sin_e.ap())
        tri_sb = const.tile([D, D], bf16, tag="tri")
        nc.sync.dma_start(tri_sb[:], tri_e.ap())
        ones_sb = const.tile([128, 128], bf16, tag="ones")
        nc.vector.memset(ones_sb[:], 1.0)
        wp_sb = const.tile([128, NQH, C], bf16, tag="wp")
        nc.sync.dma_start(wp_sb[:], wp_e.ap().rearrange("p (ko n) -> p ko n", ko=NQH))

        # persistent per-batch-pair tensors
        qT = qkvp.tile([D, NQH, TOK], bf16, tag="qT")    # rope'd, pre-scaled
        kT = qkvp.tile([D, TOK], bf16, tag="kT")         # rope'd
        vv = qkvp.tile([128, B * KB, D], bf16, tag="vv")  # token-major
        yT = qkvp.tile([D, NQH, TOK], bf16, tag="yT")    # attn out, normalized

        def rope_out(dst, src_ps, cos_ap, sin_ap):
            """dst(bf16 sbuf) = src * cos + rotate_half(src) * sin.

            The PSUM result is first cast to bf16 on the ACT engine so every
            DVE op here runs on pure 16-bit operands (2x DVE rate)."""
            sb = rtp.tile([128, NCH], bf16, tag="sb")
            nc.scalar.copy(sb[:], src_ps)
            rt = rtp.tile([128, NCH], bf16, tag="rt")
            nc.vector.tensor_scalar(out=rt[0:64, :], in0=sb[64:128, :],
                                    scalar1=-1.0, scalar2=None, op0=OP.mult)
            nc.vector.tensor_copy(out=rt[64:128, :], in_=sb[0:64, :])
            m1 = rtp.tile([128, NCH], bf16, tag="m1")
            nc.vector.tensor_tensor(out=m1[:], in0=sb[:], in1=cos_ap, op=OP.mult)
            m2 = rtp.tile([128, NCH], bf16, tag="m2")
            nc.vector.tensor_tensor(out=m2[:], in0=rt[:], in1=sin_ap, op=OP.mult)
            nc.vector.tensor_tensor(out=dst, in0=m1[:], in1=m2[:], op=OP.add)

        def emit_proj(b, qc, act_frac=2, final=False):
            # act_frac: how many of each qt-tile's 4 PSUM drains run on the
            # ACT engine (rest on DVE). Tuned per emission window so neither
            # engine outruns its exp/denominator budget. final: the very last
            # proj — both vector engines idle; drain eagerly and overlap the
            # output DMAs (issued from two queues) with remaining matmuls.
            tok0 = b * T
            for qt in range(4 * qc, 4 * qc + 4):
                osb = outp.tile([128, C], bf16, tag="osb")
                for fc in range(C // NCH):
                    ops = psum.tile([128, NCH], f32, tag="mm")
                    for kd in range(NQH):
                        nc.tensor.matmul(
                            ops[:],
                            yT[:, kd, tok0 + qt * 128: tok0 + (qt + 1) * 128],
                            wp_sb[:, kd, ts(fc, NCH)],
                            start=(kd == 0), stop=(kd == NQH - 1))
                    act_set = {0: (), 1: (0,), 2: (0, 2), 3: (0, 1, 2),
                               4: (0, 1, 2, 3)}[act_frac]
                    if fc in act_set:
                        nc.scalar.copy(osb[:, ts(fc, NCH)], ops[:])
                    else:
                        nc.vector.tensor_copy(osb[:, ts(fc, NCH)], ops[:])
                    if final:
                        eng = nc.sync if fc % 2 == 0 else nc.gpsimd
                        eng.dma_start(
                            out_e.ap()[tok0 + qt * 128: tok0 + (qt + 1) * 128,
                                       ts(fc, NCH)],
                            osb[:, ts(fc, NCH)])
                if not final:
                    nc.sync.dma_start(
                        out_e.ap()[tok0 + qt * 128: tok0 + (qt + 1) * 128, :],
                        osb[:])

        pending = None
        for b in range(B):
            tok0 = b * T
            # ---- phase 1: QKV projection + RoPE for batch b ----
            # chunk-major loads so the first projections start after ~2MB
            if b == 0:
                xt_sb = xt0_sb  # chunk 0 DMAs already emitted up top
                first_tc = 1
            else:
                xt_sb = xtp.tile([128, KT, T], bf16, tag="xt")
                first_tc = 0
            for tc_ in range(first_tc, QCH):
                for g in range(4):
                    nc.sync.dma_start(
                        xt_sb[:, 4 * g:4 * g + 4, ts(tc_, NCH)],
                        xt_r[:, 4 * g:4 * g + 4,
                             tok0 + tc_ * NCH: tok0 + (tc_ + 1) * NCH])
            for tc_ in range(QCH):
                for h in range(NQH):
                    ps = psum.tile([128, NCH], f32, tag="mm")
                    for kt in range(KT):
                        nc.tensor.matmul(ps[:],
                                         wq_sb[:, kt, h * D:(h + 1) * D],
                                         xt_sb[:, kt, ts(tc_, NCH)],
                                         start=(kt == 0), stop=(kt == KT - 1))
                    rope_out(qT[:, h, tok0 + tc_ * NCH: tok0 + (tc_ + 1) * NCH],
                             ps, cos_sb[:, ts(tc_, NCH)], sin_sb[:, ts(tc_, NCH)])
                ps = psum.tile([128, NCH], f32, tag="mm")
                for kt in range(KT):
                    nc.tensor.matmul(ps[:], wk_sb[:, kt, :],
                                     xt_sb[:, kt, ts(tc_, NCH)],
                                     start=(kt == 0), stop=(kt == KT - 1))
                rope_out(kT[:, tok0 + tc_ * NCH: tok0 + (tc_ + 1) * NCH],
                         ps, cos_sb[:, ts(tc_, NCH)], sin_sb[:, ts(tc_, NCH)])
                for ti in range(4 * tc_, 4 * tc_ + 4):
                    ps = psum.tile([128, D], f32, tag="mm")
                    for kt in range(KT):
                        nc.tensor.matmul(ps[:],
                                         xt_sb[:, kt, ti * 128:(ti + 1) * 128],
                                         wv_sb[:, kt, :],
                                         start=(kt == 0), stop=(kt == KT - 1))
                    nc.scalar.copy(vv[:, b * KB + ti, :], ps[:])

            # ---- phase 2+3: attention + out-projection for batch b ----
            # proj emission is delayed one chunk so the PE stream always has
            # the next attention chunk ahead of each proj (hides the
            # reciprocal->normalize chain on DVE).
            for qc in range(QCH):
                for h in range(NQH):
                    n_kt = 4 * qc + 4
                    yps = psum.tile([128, NCH], f32, tag="y", bufs=2)
                    den = denp.tile([128, NCH], bf16, tag="den")
                    for kti in range(n_kt):
                        dq = kti - 4 * qc
                        c0 = dq * 128 if dq > 0 else 0  # masked cols skipped
                        q_sl = qT[:, h, tok0 + qc * NCH + c0:
                                  tok0 + (qc + 1) * NCH]
                        sc = psum.tile([128, NCH], f32, tag="sc")
                        nc.tensor.matmul(sc[:, c0:],
                                         kT[:, tok0 + kti * 128: tok0 + (kti + 1) * 128],
                                         q_sl, start=True, stop=True)
                        ex = exp_p.tile([128, NCH], bf16, tag="ex")
                        nc.scalar.activation(ex[:, c0:], sc[:, c0:], AF.Exp)
                        if dq >= 0:
                            # causal mask on the (otherwise idle) Pool engine;
                            # the last diagonal runs on DVE so the denominator
                            # reduce isn't gated on a Pool round-trip
                            eng_tri = nc.vector if dq == 3 else nc.gpsimd
                            eng_tri.tensor_mul(ex[:, ts(dq, 128)],
                                               ex[:, ts(dq, 128)], tri_sb[:])
                        # denominator partial-sum on DVE (bf16, 2x rate)
                        if kti == 0:
                            nc.vector.tensor_copy(den[:], ex[:])
                        else:
                            nc.vector.tensor_tensor(out=den[:, c0:],
                                                    in0=den[:, c0:],
                                                    in1=ex[:, c0:], op=OP.add)
                        nc.tensor.matmul(yps[:, c0:], vv[:, b * KB + kti, :],
                                         ex[:, c0:],
                                         start=(kti == 0), stop=(kti == n_kt - 1))
                    # reduce denominator across partitions (one matmul), then
                    # normalize yT straight out of PSUM on the DVE.
                    dps = psum.tile([128, NCH], f32, tag="sc")
                    nc.tensor.matmul(dps[:], ones_sb[:], den[:],
                                     start=True, stop=True)
                    rec = recp.tile([128, NCH], f32, tag="rec")
                    nc.vector.reciprocal_approx_fast(rec[:], dps[:])
                    nc.vector.tensor_mul(
                        yT[:, h, tok0 + qc * NCH: tok0 + (qc + 1) * NCH],
                        yps[:], rec[:])
                if pending is not None:
                    # ACT share of PSUM drains per window: exp load grows with
                    # qc, so shift drains toward DVE in later windows.
                    emit_proj(*pending, act_frac=(2, 2, 1, 0)[qc])
                pending = (b, qc)
        emit_proj(*pending, act_frac=2, final=True)

    nc.compile()
    return nc


def _get_nc():
    if "nc" not in _COMPILED:
        _COMPILED["nc"] = _build_nc()
    return _COMPILED["nc"]


def _stage_inputs(x, Wq, Wkv, Wproj):
    xt = np.ascontiguousarray(
        x.reshape(TOK, C).T).astype(BF16)                       # [C, TOK]
    cos, sin = _rope_tables()
    cos = cos.astype(BF16)
    sin = sin.astype(BF16)
    kk, qq = np.meshgrid(np.arange(D), np.arange(D), indexing="ij")
    tri = (kk <= qq).astype(BF16)                               # [k, q]

    def pmaj(w):
        # [k, n] -> partition-major [128, (k//128)*n] so device DMAs read
        # fully contiguous rows
        k, n = w.shape
        return np.ascontiguousarray(
            w.reshape(k // 128, 128, n).transpose(1, 0, 2)
            .reshape(128, (k // 128) * n))

    in_maps = []
    for c in range(N_CORES):
        g = c // 2
        wq = pmaj((Wq[2 * c * D:(2 * c + 2) * D, :] * SCALE).T.astype(BF16))
        wk = pmaj(Wkv[g * D:(g + 1) * D, :].T.astype(BF16))
        wv = pmaj(Wkv[4 * D + g * D: 4 * D + (g + 1) * D, :].T.astype(BF16))
        wp = pmaj(Wproj[:, 2 * c * D:(2 * c + 2) * D].T.astype(BF16))
        in_maps.append({
            "xt": xt, "wq": wq, "wk": wk, "wv": wv, "wp": wp,
            "cos": cos, "sin": sin, "tri": tri,
        })
    return in_maps


def run(x, Wq, Wkv, Wproj, trace=False):
    from concourse.bass_utils import run_bass_kernel_spmd

    nc = _get_nc()
    in_maps = _stage_inputs(x, Wq, Wkv, Wproj)
    res = run_bass_kernel_spmd(nc, in_maps, core_ids=list(range(N_CORES)),
                               trace=trace)
    acc = np.zeros((TOK, C), np.float32)
    for c in range(N_CORES):
        acc += res.results[c]["out"].astype(np.float32)
    out = acc.reshape(B, T, C)
    return (out, res) if trace else (out, None)


def kernel(x, Wq, Wkv, Wproj):
    out, _ = run(np.asarray(x, np.float32), np.asarray(Wq, np.float32),
                 np.asarray(Wkv, np.float32), np.asarray(Wproj, np.float32))
    return out
